# revision 4
# baseline (speedup 1.0000x reference)
"""GPTNeoX attention QKV-split + partial RoPE + KV-cache append on 8 TRN2 cores.

Full-input contract: kernel(qkv, past_key, past_value) -> (q, k, v)
  qkv        [2, 2048, 12288] f32  (head-interleaved: per head 3*128 = [q|k|v])
  past_key   [2, 32, 2048, 128] f32
  past_value [2, 32, 2048, 128] f32
  q          [2, 32, 2048, 128] f32   (RoPE on first 32 dims)
  k, v       [2, 32, 4096, 128] f32   (past cache ++ new, k RoPE'd)

Sharding: 8 cores = 2 batches x 4 head-blocks of 8 heads (tensor-parallel over
heads + data-parallel over batch). Each core's work is independent (no
collectives). Per core the kernel is pure data movement plus a small DVE
rotary on 32 of 128 head dims.
"""

import copy

import numpy as np

import concourse.bass as bass
import concourse.mybir as mybir
from concourse.bass_utils import run_bass_kernel_spmd
from concourse.tile import TileContext

B = 2
H = 32
D = 128
ROT = 32
S = 2048
P = 2048
BASE = 10000.0

N_CORES = 8
HB = 4          # head-blocks (tensor-parallel degree per batch)
HC = H // HB    # heads per core = 8
CW = 3 * D * HC  # per-core qkv column width = 3072
NCHUNK = S // 128  # 16 row-chunks per core

F32 = mybir.dt.float32


def _hoist_extra_waits(nc):
    # This walrus build accepts only ONE sync-wait command per instruction
    # (setupSyncWait raises "Too many sync wait commands"). Tile attaches up
    # to one wait per semaphore lane to an instruction. Hoist all but the
    # last wait onto fresh same-engine NoOps inserted immediately before the
    # instruction - the engine sequencer processes them in program order, so
    # semantics are identical.
    templates = {}

    def nop_template(engine):
        if engine not in templates:
            eng = nc.engines[engine]
            bi = eng.nop()
            # remove the freshly appended nop from whatever bb it landed in
            for f in nc.m.functions:
                for bb in f.blocks:
                    if bb.instructions and bb.instructions[-1] is bi.ins:
                        bb.instructions.pop()
            templates[engine] = bi.ins
        return templates[engine]

    k = 0
    for f in nc.m.functions:
        for bb in f.blocks:
            new_list = []
            for ins in bb.instructions:
                si = ins.sync_info
                if si is not None and si.on_wait and len(si.on_wait) > 1:
                    waits = list(si.on_wait)
                    for w in waits[:-1]:
                        n = copy.deepcopy(nop_template(ins.engine))
                        n.name = f"waitnop-{k}"
                        k += 1
                        n.sync_info = mybir.SyncInfo(on_wait=[w], on_update=[])
                        new_list.append(n)
                        nc.register_instruction(n, overwrite=True)
                    si.on_wait = [waits[-1]]
                new_list.append(ins)
            bb.instructions[:] = new_list


def _rope_tables():
    # cos/sin for positions [P, P+S), first ROT/2 frequencies; layout
    # [128 partitions, chunk(16) * 16] so chunk r's table is cols r*16:(r+1)*16.
    inv_freq = 1.0 / (BASE ** (np.arange(0, ROT, 2, dtype=np.float32) / np.float32(ROT)))
    pos = np.arange(P, P + S, dtype=np.float32)
    freqs = pos[:, None] * inv_freq[None, :].astype(np.float32)  # [S, 16]
    cos = np.cos(freqs).astype(np.float32)
    sin = np.sin(freqs).astype(np.float32)
    # [S,16] -> [chunk, part, 16] -> [part, chunk, 16] -> [128, 256]
    cos_t = np.ascontiguousarray(cos.reshape(NCHUNK, 128, 16).transpose(1, 0, 2).reshape(128, NCHUNK * 16))
    sin_t = np.ascontiguousarray(sin.reshape(NCHUNK, 128, 16).transpose(1, 0, 2).reshape(128, NCHUNK * 16))
    return cos_t, sin_t


def _build(xt_bufs=4, past_split=4):
    nc = bass.Bass()
    qkv = nc.dram_tensor("qkv_s", [S, CW], F32, kind="ExternalInput")
    pk = nc.dram_tensor("pk", [HC, P, D], F32, kind="ExternalInput")
    pv = nc.dram_tensor("pv", [HC, P, D], F32, kind="ExternalInput")
    cos_in = nc.dram_tensor("cos_t", [128, NCHUNK * 16], F32, kind="ExternalInput")
    sin_in = nc.dram_tensor("sin_t", [128, NCHUNK * 16], F32, kind="ExternalInput")
    q_o = nc.dram_tensor("q_o", [HC, S, D], F32, kind="ExternalOutput")
    k_o = nc.dram_tensor("k_o", [HC, P + S, D], F32, kind="ExternalOutput")
    v_o = nc.dram_tensor("v_o", [HC, P + S, D], F32, kind="ExternalOutput")

    # Past-cache bounce geometry: pk/pv are [HC, P, D] = 16384 rows x 512B,
    # viewed as 128 partitions x 128 rows. Each bounce tile moves `c` rows per
    # partition (c*512B contiguous descriptors on both DRAM sides; all 16 SDMA
    # engines engaged since SBUF is involved). A direct DRAM->DRAM DMA lands
    # on only 2 SDMA engines, which serializes ~310us - hence the bounce.
    PB_C = 32                       # rows per partition per bounce tile
    PB_N = (P * HC // 128) // PB_C  # bounce tiles per tensor = 4
    PB_W = PB_C * D                 # free width = 4096

    def past_views(dram_past, dram_out):
        src = dram_past.rearrange("h (pp ci c) d -> h pp ci (c d)", pp=16, ci=PB_N, c=PB_C)
        dst = dram_out[:, 0:P, :].rearrange(
            "h (pp ci c) d -> h pp ci (c d)", pp=16, ci=PB_N, c=PB_C
        )
        return src, dst

    with TileContext(nc) as tc:
        with (
            tc.tile_pool(name="tab", bufs=1) as tabs,
            tc.tile_pool(name="x", bufs=xt_bufs) as xp,
            tc.tile_pool(name="tmp", bufs=4) as tp,
            tc.tile_pool(name="pb", bufs=4) as pbp,
        ):
            cos_sb = tabs.tile([128, NCHUNK * 16], F32, tag="cos")
            nc.sync.dma_start(out=cos_sb[:], in_=cos_in[:])
            sin_sb = tabs.tile([128, NCHUNK * 16], F32, tag="sin")
            nc.sync.dma_start(out=sin_sb[:], in_=sin_in[:])

            pk_src, pk_dst = past_views(pk, k_o)
            pv_src, pv_dst = past_views(pv, v_o)
            past_work = []
            for i in range(PB_N):
                past_work.append((pk_src, pk_dst, i))
                past_work.append((pv_src, pv_dst, i))

            def emit_past(job):
                src, dst, i = job
                bt = pbp.tile([128, PB_W], F32, tag="pb")
                nc.scalar.dma_start(out=bt[:], in_=src[:, :, i, :])
                nc.scalar.dma_start(out=dst[:, :, i, :], in_=bt[:])

            for r in range(NCHUNK):
                s0 = r * 128
                xt = xp.tile([128, CW], F32, tag="xt")
                nc.scalar.dma_start(out=xt[:], in_=qkv[s0 : s0 + 128, :])
                x4 = xt[:].rearrange("p (h t d) -> p h t d", h=HC, t=3)

                cos_ap = cos_sb[:, r * 16 : (r + 1) * 16].unsqueeze(1).broadcast_to((128, HC, 16))
                sin_ap = sin_sb[:, r * 16 : (r + 1) * 16].unsqueeze(1).broadcast_to((128, HC, 16))

                for t in range(2):  # 0 = q, 1 = k
                    x1 = x4[:, :, t, 0:16]
                    x2 = x4[:, :, t, 16:32]
                    ta = tp.tile([128, HC * 16], F32, tag="ta")
                    tb = tp.tile([128, HC * 16], F32, tag="tb")
                    tcc = tp.tile([128, HC * 16], F32, tag="tc")
                    td = tp.tile([128, HC * 16], F32, tag="td")
                    ta_ap = ta[:].rearrange("p (h j) -> p h j", h=HC)
                    tb_ap = tb[:].rearrange("p (h j) -> p h j", h=HC)
                    tc_ap = tcc[:].rearrange("p (h j) -> p h j", h=HC)
                    td_ap = td[:].rearrange("p (h j) -> p h j", h=HC)
                    nc.vector.tensor_mul(ta_ap, x1, cos_ap)
                    nc.vector.tensor_mul(tb_ap, x2, sin_ap)
                    nc.vector.tensor_mul(tc_ap, x2, cos_ap)
                    nc.vector.tensor_mul(td_ap, x1, sin_ap)
                    nc.vector.tensor_sub(x1, ta_ap, tb_ap)
                    nc.vector.tensor_add(x2, tc_ap, td_ap)

                nc.sync.dma_start(
                    out=q_o[:, s0 : s0 + 128, :].transpose([1, 0, 2]), in_=x4[:, :, 0, :]
                )
                nc.scalar.dma_start(
                    out=k_o[:, P + s0 : P + s0 + 128, :].transpose([1, 0, 2]), in_=x4[:, :, 1, :]
                )
                nc.sync.dma_start(
                    out=v_o[:, P + s0 : P + s0 + 128, :].transpose([1, 0, 2]), in_=x4[:, :, 2, :]
                )
                # interleave one past-bounce job every other chunk
                if r % 2 == 0 and past_work:
                    emit_past(past_work.pop(0))
            while past_work:
                emit_past(past_work.pop(0))

    _hoist_extra_waits(nc)
    return nc


_NC_CACHE = None


def _get_nc():
    global _NC_CACHE
    if _NC_CACHE is None:
        _NC_CACHE = _build()
    return _NC_CACHE


def kernel(qkv, past_key, past_value):
    qkv = np.asarray(qkv, dtype=np.float32)
    past_key = np.asarray(past_key, dtype=np.float32)
    past_value = np.asarray(past_value, dtype=np.float32)

    cos_t, sin_t = _rope_tables()
    nc = _get_nc()

    in_maps = []
    for c in range(N_CORES):
        b, hb = divmod(c, HB)
        h0 = hb * HC
        in_maps.append(
            {
                "qkv_s": np.ascontiguousarray(qkv[b, :, hb * CW : (hb + 1) * CW]),
                "pk": np.ascontiguousarray(past_key[b, h0 : h0 + HC]),
                "pv": np.ascontiguousarray(past_value[b, h0 : h0 + HC]),
                "cos_t": cos_t,
                "sin_t": sin_t,
            }
        )

    res = run_bass_kernel_spmd(nc, in_maps, list(range(N_CORES)))

    q = np.empty((B, H, S, D), np.float32)
    k = np.empty((B, H, P + S, D), np.float32)
    v = np.empty((B, H, P + S, D), np.float32)
    for c in range(N_CORES):
        b, hb = divmod(c, HB)
        h0 = hb * HC
        q[b, h0 : h0 + HC] = res.results[c]["q_o"]
        k[b, h0 : h0 + HC] = res.results[c]["k_o"]
        v[b, h0 : h0 + HC] = res.results[c]["v_o"]
    return q, k, v


# revision 5
# speedup vs baseline: 1.0517x; 1.0517x over previous
"""GPTNeoX attention QKV-split + partial RoPE + KV-cache append on 8 TRN2 cores.

Full-input contract: kernel(qkv, past_key, past_value) -> (q, k, v)
  qkv        [2, 2048, 12288] f32  (head-interleaved: per head 3*128 = [q|k|v])
  past_key   [2, 32, 2048, 128] f32
  past_value [2, 32, 2048, 128] f32
  q          [2, 32, 2048, 128] f32   (RoPE on first 32 dims)
  k, v       [2, 32, 4096, 128] f32   (past cache ++ new, k RoPE'd)

Sharding: 8 cores = 2 batches x 4 head-blocks of 8 heads (tensor-parallel over
heads + data-parallel over batch). Each core's work is independent (no
collectives). Per core the kernel is pure data movement plus a small DVE
rotary on 32 of 128 head dims.
"""

import copy

import numpy as np

import concourse.bass as bass
import concourse.mybir as mybir
from concourse.bass_utils import run_bass_kernel_spmd
from concourse.tile import TileContext

B = 2
H = 32
D = 128
ROT = 32
S = 2048
P = 2048
BASE = 10000.0

N_CORES = 8
HB = 4          # head-blocks (tensor-parallel degree per batch)
HC = H // HB    # heads per core = 8
CW = 3 * D * HC  # per-core qkv column width = 3072
NCHUNK = S // 128  # 16 row-chunks per core

F32 = mybir.dt.float32


def _hoist_extra_waits(nc):
    # This walrus build accepts only ONE sync-wait command per instruction
    # (setupSyncWait raises "Too many sync wait commands"). Tile attaches up
    # to one wait per semaphore lane to an instruction. Hoist all but the
    # last wait onto fresh same-engine NoOps inserted immediately before the
    # instruction - the engine sequencer processes them in program order, so
    # semantics are identical.
    templates = {}

    def nop_template(engine):
        if engine not in templates:
            eng = nc.engines[engine]
            bi = eng.nop()
            # remove the freshly appended nop from whatever bb it landed in
            for f in nc.m.functions:
                for bb in f.blocks:
                    if bb.instructions and bb.instructions[-1] is bi.ins:
                        bb.instructions.pop()
            templates[engine] = bi.ins
        return templates[engine]

    k = 0
    for f in nc.m.functions:
        for bb in f.blocks:
            new_list = []
            for ins in bb.instructions:
                si = ins.sync_info
                if si is not None and si.on_wait and len(si.on_wait) > 1:
                    waits = list(si.on_wait)
                    for w in waits[:-1]:
                        n = copy.deepcopy(nop_template(ins.engine))
                        n.name = f"waitnop-{k}"
                        k += 1
                        n.sync_info = mybir.SyncInfo(on_wait=[w], on_update=[])
                        new_list.append(n)
                        nc.register_instruction(n, overwrite=True)
                    si.on_wait = [waits[-1]]
                new_list.append(ins)
            bb.instructions[:] = new_list


def _rope_tables():
    # cos/sin for positions [P, P+S), first ROT/2 frequencies; layout
    # [128 partitions, chunk(16) * 16] so chunk r's table is cols r*16:(r+1)*16.
    inv_freq = 1.0 / (BASE ** (np.arange(0, ROT, 2, dtype=np.float32) / np.float32(ROT)))
    pos = np.arange(P, P + S, dtype=np.float32)
    freqs = pos[:, None] * inv_freq[None, :].astype(np.float32)  # [S, 16]
    cos = np.cos(freqs).astype(np.float32)
    sin = np.sin(freqs).astype(np.float32)
    # [S,16] -> [chunk, part, 16] -> [part, chunk, 16] -> [128, 256]
    cos_t = np.ascontiguousarray(cos.reshape(NCHUNK, 128, 16).transpose(1, 0, 2).reshape(128, NCHUNK * 16))
    sin_t = np.ascontiguousarray(sin.reshape(NCHUNK, 128, 16).transpose(1, 0, 2).reshape(128, NCHUNK * 16))
    return cos_t, sin_t


def _build(xt_bufs=4, past_split=4):
    nc = bass.Bass()
    qkv = nc.dram_tensor("qkv_s", [S, CW], F32, kind="ExternalInput")
    pk = nc.dram_tensor("pk", [HC, P, D], F32, kind="ExternalInput")
    pv = nc.dram_tensor("pv", [HC, P, D], F32, kind="ExternalInput")
    cos_in = nc.dram_tensor("cos_t", [128, NCHUNK * 16], F32, kind="ExternalInput")
    sin_in = nc.dram_tensor("sin_t", [128, NCHUNK * 16], F32, kind="ExternalInput")
    q_o = nc.dram_tensor("q_o", [HC, S, D], F32, kind="ExternalOutput")
    k_o = nc.dram_tensor("k_o", [HC, P + S, D], F32, kind="ExternalOutput")
    v_o = nc.dram_tensor("v_o", [HC, P + S, D], F32, kind="ExternalOutput")

    # Past-cache bounce geometry: pk/pv are [HC, P, D] = 16384 rows x 512B,
    # viewed as 128 partitions x 128 rows. Each bounce tile moves `c` rows per
    # partition (c*512B contiguous descriptors on both DRAM sides; all 16 SDMA
    # engines engaged since SBUF is involved). A direct DRAM->DRAM DMA lands
    # on only 2 SDMA engines, which serializes ~310us - hence the bounce.
    PB_C = 16                       # rows per partition per bounce tile
    PB_N = (P * HC // 128) // PB_C  # bounce tiles per tensor = 4
    PB_W = PB_C * D                 # free width = 4096

    def past_views(dram_past, dram_out):
        src = dram_past.rearrange("h (pp ci c) d -> h pp ci (c d)", pp=16, ci=PB_N, c=PB_C)
        dst = dram_out[:, 0:P, :].rearrange(
            "h (pp ci c) d -> h pp ci (c d)", pp=16, ci=PB_N, c=PB_C
        )
        return src, dst

    with TileContext(nc) as tc:
        with (
            tc.tile_pool(name="tab", bufs=1) as tabs,
            tc.tile_pool(name="x", bufs=xt_bufs) as xp,
            tc.tile_pool(name="tmp", bufs=4) as tp,
            tc.tile_pool(name="pb", bufs=4) as pbp,
        ):
            cos_sb = tabs.tile([128, NCHUNK * 16], F32, tag="cos")
            nc.sync.dma_start(out=cos_sb[:], in_=cos_in[:])
            sin_sb = tabs.tile([128, NCHUNK * 16], F32, tag="sin")
            nc.sync.dma_start(out=sin_sb[:], in_=sin_in[:])

            pk_src, pk_dst = past_views(pk, k_o)
            pv_src, pv_dst = past_views(pv, v_o)
            past_work = []
            for i in range(PB_N):
                past_work.append((pk_src, pk_dst, i))
                past_work.append((pv_src, pv_dst, i))

            def emit_past(job):
                src, dst, i = job
                bt = pbp.tile([128, PB_W], F32, tag="pb")
                nc.scalar.dma_start(out=bt[:], in_=src[:, :, i, :])
                nc.scalar.dma_start(out=dst[:, :, i, :], in_=bt[:])

            for r in range(NCHUNK):
                s0 = r * 128
                xt = xp.tile([128, CW], F32, tag="xt")
                nc.scalar.dma_start(out=xt[:], in_=qkv[s0 : s0 + 128, :])
                x4 = xt[:].rearrange("p (h t d) -> p h t d", h=HC, t=3)

                cos_ap = cos_sb[:, r * 16 : (r + 1) * 16].unsqueeze(1).broadcast_to((128, HC, 16))
                sin_ap = sin_sb[:, r * 16 : (r + 1) * 16].unsqueeze(1).broadcast_to((128, HC, 16))

                for t in range(2):  # 0 = q, 1 = k
                    x1 = x4[:, :, t, 0:16]
                    x2 = x4[:, :, t, 16:32]
                    ta = tp.tile([128, HC * 16], F32, tag="ta")
                    tb = tp.tile([128, HC * 16], F32, tag="tb")
                    tcc = tp.tile([128, HC * 16], F32, tag="tc")
                    td = tp.tile([128, HC * 16], F32, tag="td")
                    ta_ap = ta[:].rearrange("p (h j) -> p h j", h=HC)
                    tb_ap = tb[:].rearrange("p (h j) -> p h j", h=HC)
                    tc_ap = tcc[:].rearrange("p (h j) -> p h j", h=HC)
                    td_ap = td[:].rearrange("p (h j) -> p h j", h=HC)
                    nc.vector.tensor_mul(ta_ap, x1, cos_ap)
                    nc.vector.tensor_mul(tb_ap, x2, sin_ap)
                    nc.vector.tensor_mul(tc_ap, x2, cos_ap)
                    nc.vector.tensor_mul(td_ap, x1, sin_ap)
                    nc.vector.tensor_sub(x1, ta_ap, tb_ap)
                    nc.vector.tensor_add(x2, tc_ap, td_ap)

                nc.sync.dma_start(
                    out=q_o[:, s0 : s0 + 128, :].transpose([1, 0, 2]), in_=x4[:, :, 0, :]
                )
                nc.scalar.dma_start(
                    out=k_o[:, P + s0 : P + s0 + 128, :].transpose([1, 0, 2]), in_=x4[:, :, 1, :]
                )
                nc.sync.dma_start(
                    out=v_o[:, P + s0 : P + s0 + 128, :].transpose([1, 0, 2]), in_=x4[:, :, 2, :]
                )
                # interleave one past-bounce job every other chunk
                if r % 2 == 0 and past_work:
                    emit_past(past_work.pop(0))
            while past_work:
                emit_past(past_work.pop(0))

    _hoist_extra_waits(nc)
    return nc


_NC_CACHE = None


def _get_nc():
    global _NC_CACHE
    if _NC_CACHE is None:
        _NC_CACHE = _build()
    return _NC_CACHE


def kernel(qkv, past_key, past_value):
    qkv = np.asarray(qkv, dtype=np.float32)
    past_key = np.asarray(past_key, dtype=np.float32)
    past_value = np.asarray(past_value, dtype=np.float32)

    cos_t, sin_t = _rope_tables()
    nc = _get_nc()

    in_maps = []
    for c in range(N_CORES):
        b, hb = divmod(c, HB)
        h0 = hb * HC
        in_maps.append(
            {
                "qkv_s": np.ascontiguousarray(qkv[b, :, hb * CW : (hb + 1) * CW]),
                "pk": np.ascontiguousarray(past_key[b, h0 : h0 + HC]),
                "pv": np.ascontiguousarray(past_value[b, h0 : h0 + HC]),
                "cos_t": cos_t,
                "sin_t": sin_t,
            }
        )

    res = run_bass_kernel_spmd(nc, in_maps, list(range(N_CORES)))

    q = np.empty((B, H, S, D), np.float32)
    k = np.empty((B, H, P + S, D), np.float32)
    v = np.empty((B, H, P + S, D), np.float32)
    for c in range(N_CORES):
        b, hb = divmod(c, HB)
        h0 = hb * HC
        q[b, h0 : h0 + HC] = res.results[c]["q_o"]
        k[b, h0 : h0 + HC] = res.results[c]["k_o"]
        v[b, h0 : h0 + HC] = res.results[c]["v_o"]
    return q, k, v
